# revision 1
# baseline (speedup 1.0000x reference)
"""BQuantConv1d Trainium2 kernel.

Math: the reference's per-token LUT + gather is algebraically a matmul:
  out[n, f] = sum_i x[n, i] * W[i, f] + bias[f]
  W[8g+j, f] = sum_b scale[b, f] * (2*bit_{7-j}(binary[b, g, f]) - 1)

Sharding: 2 token-groups x 4 f-groups over 8 cores, no collectives
(host slices inputs / concatenates outputs; layout-only host work).
Contraction order is permuted to i' = j*128 + g (host permutes xT rows to
match) so each decoded weight chunk j lands on contiguous partitions.

Per core:
  - decode W'(1024, 256) from int16 codes with a sign-bit trick:
    W element = +-scale[b, f] exactly, built by XORing the fp16 scale's
    sign bit (scales arrive sign-pre-flipped) with the masked quant bit
    (c << (8+j)) & 0x8000, as int32 SWAR on DVE (walrus allows bitvec
    ops only there, 32-bit only); b-reduction is an fp16 add tree;
  - outT[f_shard, n_shard] = W'.T @ xT on the PE in fp16, accumulating
    the 8 contraction chunks across 8 concurrent PSUM banks (f32),
    W-chunk-outer so the PE consumes each chunk as it is decoded; each
    PSUM bank is seeded with the bias via a K=1 bias x ones matmul;
  - PSUM copied out on ACT/DVE (fp16), output DMAs spread over the
    sync/scalar/gpsimd queues, one contiguous DRAM block per tile.
"""

import numpy as np

try:
    import concourse.bass as bass  # noqa: F401
except ImportError:
    import sys

    sys.path.insert(0, "/opt/trn_rl_repo")
    import concourse.bass as bass  # noqa: F401

import concourse.bacc as bacc
import concourse.mybir as mybir
import concourse.tile as tile

B, T, NX, NF = 2, 2048, 1024, 1024
N_TOK = B * T
BITS = 8
G = NX // 8  # 128 code groups
PT, PF = 2, 4  # token-parallel x feature-parallel
TOK = N_TOK // PT  # tokens per core
NFS = NF // PF  # output features per core
P = 128
MM_N = 512  # moving free dim per matmul

AX = mybir.AxisListType
OP = mybir.AluOpType
F32 = mybir.dt.float32
BF16 = mybir.dt.float16  # compute dtype (fp16: same SWAR, more mantissa)
I16 = mybir.dt.int16
I32 = mybir.dt.int32
ACT_F = mybir.ActivationFunctionType
BF16NP = np.float16


def build_graph(nc, tok=TOK, nfs=NFS, pair_groups=None):
    """pair_groups: replica groups of size 2 for the W AllGather (each
    member decodes one 128-column half of the shared W shard).  None =
    no collective; each core decodes its full W shard."""
    nfb = nfs // P  # f blocks of 128
    nch = tok // MM_N  # moving chunks
    nfd = nfs // 2 if pair_groups else nfs  # decoded columns per core
    xt_d = nc.dram_tensor("xt", (8, P, tok), BF16, kind="ExternalInput")
    cd_d = nc.dram_tensor("codes", (P, 8 * nfd), I16, kind="ExternalInput")
    sc_d = nc.dram_tensor("scales", (P, 8 * nfd), BF16, kind="ExternalInput")
    bi_d = nc.dram_tensor("biasv", (nfs,), F32, kind="ExternalInput")
    out_d = nc.dram_tensor("out", (nfb, nch, P, MM_N), BF16, kind="ExternalOutput")

    with tile.TileContext(nc) as tc:
        with (
            tc.tile_pool(name="xp", bufs=8) as xp,
            tc.tile_pool(name="cp", bufs=8) as cp,
            tc.tile_pool(name="wp", bufs=8) as wp,
            tc.tile_pool(name="qp", bufs=3) as qp,
            tc.tile_pool(name="cst", bufs=1) as cst,
            tc.tile_pool(name="op", bufs=8) as op_,
            tc.tile_pool(name="pp", bufs=8, space="PSUM") as pp,
            tc.tile_pool(name="dr", bufs=8, space="DRAM") as dr,
        ):
            # --- loads; codes first: decode is the critical path ---
            # codes tile: partition = g, free = (b, f); host pre-arranged
            cd = cp.tile([P, 8 * nfd], I16, tag="cd")
            nc.sync.dma_start(cd[:], cd_d[:])
            sc_bc = cst.tile([P, 8 * nfd], BF16, tag="sc_bc")
            nc.sync.dma_start(sc_bc[:], sc_d[:])
            bi_f32 = cst.tile([1, nfs], F32, tag="bi_f32")
            nc.sync.dma_start(bi_f32[:], bi_d.rearrange("(o f) -> o f", o=1))
            bi_row = cst.tile([1, nfs], BF16, tag="bi_row")
            nc.scalar.copy(bi_row[:], bi_f32[:])
            ones = cst.tile([1, MM_N], BF16, tag="ones")
            nc.vector.memset(ones[:], 1.0)
            xts = []
            for j in range(8):
                xt = xp.tile([P, tok], BF16, tag="xt")
                nc.sync.dma_start(xt[:], xt_d[j])
                xts.append(xt)

            # --- decode W chunks ---
            # Sign-bit trick: masked quant bit (inverted) XORed onto the
            # fp16 scale's sign gives +-scale exactly.  Bitvec ops are
            # DVE-only and 32-bit-only on walrus, so they run as int32 SWAR
            # over int16-lane pairs: a left shift by 8+j sources each
            # lane's bit 15 from within the same lane, and the 0x80008000
            # mask keeps only the two sign bits.  The bit inversion is
            # folded into a one-time sign-flip of the scale tile:
            #   ((c << (8+j)) & M) ^ (sc ^ M)  ==  ((~c << (8+j)) & M) ^ sc
            MSK = -2147450880  # 0x80008000 as int32
            sc_ng = sc_bc  # host passes scales negated (sign pre-flipped)
            ws = []
            for j in range(8):
                sg = qp.tile([P, 8 * nfd], I16, tag="sg")
                nc.vector.tensor_scalar(
                    sg[:].bitcast(I32), cd[:].bitcast(I32), 8 + j, MSK,
                    OP.logical_shift_left, OP.bitwise_and,
                )
                wsg = qp.tile([P, 8 * nfd], I16, tag="wsg")
                nc.vector.tensor_tensor(
                    wsg[:].bitcast(I32), sg[:].bitcast(I32),
                    sc_ng[:].bitcast(I32), OP.bitwise_xor,
                )
                # b-reduction as an fp16 add tree (all DVE: a GPSIMD tree
                # adds ~3us latency to its chunk's critical path)
                teng = nc.vector
                wv = wsg[:].bitcast(BF16)
                h1 = qp.tile([P, 4 * nfd], BF16, tag="h1")
                teng.tensor_tensor(
                    h1[:], wv[:, : 4 * nfd], wv[:, 4 * nfd :], OP.add
                )
                h2 = qp.tile([P, 2 * nfd], BF16, tag="h2")
                teng.tensor_tensor(
                    h2[:], h1[:, : 2 * nfd], h1[:, 2 * nfd :], OP.add
                )
                if not pair_groups:
                    w = wp.tile([P, nfs], BF16, tag="w")
                    teng.tensor_tensor(w[:], h2[:, :nfs], h2[:, nfs:], OP.add)
                    ws.append(w)
                    continue
                # pair-dedup: this core decoded one 128-col half; exchange
                # with the partner core that shares the same f-shard.
                wown = wp.tile([P, nfd], BF16, tag="wown", name=f"wown{j}")
                teng.tensor_tensor(wown[:], h2[:, :nfd], h2[:, nfd:], OP.add)
                agin = dr.tile([P, nfd], BF16, tag="agin", name=f"agin{j}")
                nc.sync.dma_start(agin[:], wown[:])
                agout = dr.tile([2, P, nfd], BF16, tag="agout", name=f"agout{j}")
                nc.gpsimd.collective_compute(
                    "AllGather",
                    mybir.AluOpType.bypass,
                    replica_groups=pair_groups,
                    ins=[agin.opt()],
                    outs=[agout.opt()],
                )
                w = wp.tile([P, nfs], BF16, tag="w", name=f"w{j}")
                nc.sync.dma_start(
                    w[:].rearrange("p (c f) -> p c f", c=2),
                    agout[:].rearrange("c p f -> p c f"),
                )
                ws.append(w)

            # --- matmul: outT[f, n] = bias + sum_j W_j.T @ xT_j ---
            # j outermost: each W chunk feeds the PE as soon as it is
            # decoded, all nfb*nch PSUM banks accumulate concurrently.
            # The last chunk (j=7) is issued group-by-group so evacuation
            # and output DMA overlap the remaining j=7 matmuls.
            pss = {}
            for fb in range(nfb):
                for ch in range(nch):
                    pss[(fb, ch)] = pp.tile(
                        [P, MM_N], F32, tag="ps", name=f"ps{fb}_{ch}"
                    )
                    # seed the accumulator with the bias via a K=1 matmul:
                    # bias_row.T @ ones = bias broadcast along n
                    nc.tensor.matmul(
                        pss[(fb, ch)][:],
                        bi_row[:, fb * P : (fb + 1) * P],
                        ones[:],
                        start=True,
                        stop=False,
                    )
            for j in range(6):
                for fb in range(nfb):
                    for ch in range(nch):
                        nc.tensor.matmul(
                            pss[(fb, ch)][:],
                            ws[j][:, fb * P : (fb + 1) * P],
                            xts[j][:, ch * MM_N : (ch + 1) * MM_N],
                            start=False,
                            stop=False,
                        )
            for fb in range(nfb):
                for ch in range(nch):
                    for jl in (6, 7):
                        nc.tensor.matmul(
                            pss[(fb, ch)][:],
                            ws[jl][:, fb * P : (fb + 1) * P],
                            xts[jl][:, ch * MM_N : (ch + 1) * MM_N],
                            start=False,
                            stop=(jl == 7),
                        )
                    ob = op_.tile([P, MM_N], BF16, tag="ob")
                    if ch % 2:
                        nc.vector.tensor_copy(ob[:], pss[(fb, ch)][:])
                    else:
                        nc.scalar.copy(ob[:], pss[(fb, ch)][:])
                    deng = (nc.sync, nc.gpsimd, nc.scalar)[ch % 3]
                    deng.dma_start(out_d[fb, ch], ob[:])
    nc.compile()
    return nc


_I_PERM = 8 * (np.arange(NX) % G) + np.arange(NX) // G  # i' -> i


PAIR_GROUPS = [[pf, PF + pf] for pf in range(PF)]


def host_prep(x, binary, scale, bias, pair=True):
    """Layout-only sharding (plus x's bf16 compute-precision cast).
    Returns in_maps for cores 0..7 (pt = c//PF, pf = c%PF).  With
    pair=True each core gets only the 128-column half of codes/scales it
    decodes (the partner core supplies the other half via AllGather)."""
    x2 = np.ascontiguousarray(x.reshape(N_TOK, NX).T)[_I_PERM]  # (NX, N)
    x2 = x2.astype(BF16NP)  # compute dtype
    binary16 = binary.astype(np.int16)  # lossless: codes are 0..255
    nfd = NFS // 2 if pair else NFS
    in_maps = []
    for c in range(8):
        pt, pf = c // PF, c % PF
        f0 = pf * NFS + (pt * nfd if pair else 0)
        xs = np.ascontiguousarray(x2[:, pt * TOK : (pt + 1) * TOK]).reshape(
            8, P, TOK
        )
        cs = np.ascontiguousarray(
            binary16[:, :, f0 : f0 + nfd].transpose(1, 0, 2)
        ).reshape(P, 8 * nfd)
        ss = np.ascontiguousarray(
            np.broadcast_to(
                (-scale[:, f0 : f0 + nfd].astype(BF16NP)).reshape(1, 8 * nfd),
                (P, 8 * nfd),
            )
        )
        bs = np.ascontiguousarray(bias[pf * NFS : (pf + 1) * NFS])
        in_maps.append({"xt": xs, "codes": cs, "scales": ss, "biasv": bs})
    return in_maps


def host_assemble(results):
    """results[c]["out"]: (NFB, 128, TOK) -> full (B, T, NF)."""
    outT = np.empty((NF, N_TOK), dtype=np.float32)
    for c in range(8):
        pt, pf = c // PF, c % PF
        o = np.asarray(results[c]["out"], dtype=np.float32)
        # (nfb, nch, P, MM_N) -> (NFS, TOK)
        o = o.transpose(0, 2, 1, 3).reshape(NFS, TOK)
        outT[pf * NFS : (pf + 1) * NFS, pt * TOK : (pt + 1) * TOK] = o
    return np.ascontiguousarray(outT.T).reshape(B, T, NF)


_NC_CACHE = {}


def _get_nc(pair=True):
    key = ("nc", pair)
    if key not in _NC_CACHE:
        nc = bacc.Bacc(None, target_bir_lowering=False)
        build_graph(nc, pair_groups=PAIR_GROUPS if pair else None)
        _NC_CACHE[key] = nc
    return _NC_CACHE[key]


def kernel(**inputs):
    from concourse.bass_utils import run_bass_kernel_spmd

    inputs = {k: np.asarray(v) for k, v in inputs.items()}
    # Pair-dedup via AllGather halves decode work but MultiCoreSim prices
    # the 8 tiny collectives at far more than the ~10us saved; keep off.
    pair = False
    in_maps = host_prep(
        inputs["x"], inputs["binary"], inputs["scale"], inputs["bias"], pair=pair
    )
    res = run_bass_kernel_spmd(_get_nc(pair), in_maps, core_ids=list(range(8)))
    return host_assemble(res.results)



# revision 18
# speedup vs baseline: 1.0392x; 1.0392x over previous
"""BQuantConv1d Trainium2 kernel.

Math: the reference's per-token LUT + gather is algebraically a matmul:
  out[n, f] = sum_i x[n, i] * W[i, f] + bias[f]
  W[8g+j, f] = sum_b scale[b, f] * (2*bit_{7-j}(binary[b, g, f]) - 1)

Sharding: 2 token-groups x 4 f-groups over 8 cores, no collectives
(host slices inputs / concatenates outputs; layout-only host work).
Contraction order is permuted to i' = j*128 + g (host permutes xT rows
to match) so each decoded weight chunk j lands on contiguous partitions.

Per core:
  - decode W'(1024, 256) from int16 codes with a sign-bit trick:
    W element = +-scale[b, f] exactly, built by XORing the fp16 scale's
    sign bit (scales arrive sign-pre-flipped) with the masked quant bit
    (c << (8+j)) & 0x8000, as int32 SWAR (bitvec ops are DVE-only and
    32-bit-only on walrus); the fp16 b-reduction add-trees of the first
    three chunks run on the otherwise-idle GPSIMD (Pool) engine.
  - outT[f_shard, n_shard] = W'.T @ xT on the PE in fp16 across 8
    concurrent PSUM banks (f32), batches emitted in decode-completion
    order.  Zero-contribution pacer matmuls (junk stationary x zeroed
    moving) keep the PE p-state ramp warm through the decode phase —
    idle-gap-dependent throttling otherwise runs matmuls 2-4x slower.
  - bias is folded into the PSUM evacuation (ACT activation-bias /
    DVE tensor_scalar-bias, per-partition operands); the two
    Pool-evacuated tiles get a K=1 bias x ones seed matmul instead.
  - output DMAs are placed per-queue strictly after that queue's other
    work (in-order queues: a waiting head blocks everything behind it).
"""

import numpy as np

try:
    import concourse.bass as bass  # noqa: F401
except ImportError:
    import sys

    sys.path.insert(0, "/opt/trn_rl_repo")
    import concourse.bass as bass  # noqa: F401

import concourse.bacc as bacc
import concourse.mybir as mybir
import concourse.tile as tile
from concourse.tile_rust import add_dep_helper

B, T, NX, NF = 2, 2048, 1024, 1024
N_TOK = B * T
BITS = 8
G = NX // 8  # 128 code groups
PT, PF = 2, 4  # token-parallel x feature-parallel
TOK = N_TOK // PT  # tokens per core
NFS = NF // PF  # output features per core
P = 128
MM_N = 512  # moving free dim per matmul

AX = mybir.AxisListType
OP = mybir.AluOpType
F32 = mybir.dt.float32
F16 = mybir.dt.float16  # compute dtype (fp16: sign-bit SWAR + more mantissa)
I16 = mybir.dt.int16
I32 = mybir.dt.int32
ACT_F = mybir.ActivationFunctionType
F16NP = np.float16

MSK = -2147450880  # 0x80008000 as int32

POOL_TREES = (1, 2, 3)  # chunks whose b-reduction tree runs on Pool
BATCH_ORDER = (0, 1, 4, 2, 5, 6, 3, 7)  # ~ decode completion order
EVAC = ("act", "dve", "act", "dve", "act", "dve", "act", "dve")
DMA_Q = ("sp", "sp", "act", "pool")  # one queue per tile-pair DMA


def build_graph(nc, tok=TOK, nfs=NFS):
    nfb = nfs // P  # f blocks of 128
    nch = tok // MM_N  # moving chunks
    xt_d = nc.dram_tensor("xt", (8, P, tok), F16, kind="ExternalInput")
    cd_d = nc.dram_tensor("codes", (P, 8 * nfs), I16, kind="ExternalInput")
    sc_d = nc.dram_tensor("scales", (P, 8 * nfs), F16, kind="ExternalInput")
    b2_d = nc.dram_tensor("biasc", (P, nfs // P), F32, kind="ExternalInput")
    out_d = nc.dram_tensor("out", (nfb, nch, P, MM_N), F16, kind="ExternalOutput")

    with tile.TileContext(nc) as tc:
        with (
            tc.tile_pool(name="xp", bufs=8) as xp,
            tc.tile_pool(name="cp", bufs=1) as cp,
            tc.tile_pool(name="wp", bufs=8) as wp,
            tc.tile_pool(name="qp", bufs=8) as qp,
            tc.tile_pool(name="cst", bufs=1) as cst,
            tc.tile_pool(name="op", bufs=8) as op_,
            tc.tile_pool(name="pp", bufs=8, space="PSUM") as pp,
        ):
            # --- loads; codes first: decode is the critical path ---
            cd = cp.tile([P, 8 * nfs], I16, tag="cd")
            nc.sync.dma_start(cd[:], cd_d[:])
            sc_bc = cst.tile([P, 8 * nfs], F16, tag="sc_bc")
            nc.scalar.dma_start(sc_bc[:], sc_d[:])
            bi2 = cst.tile([P, nfb], F32, tag="bi2")
            nc.scalar.dma_start(bi2[:], b2_d[:])
            # PE pacer scratch (zeroed moving makes pacer matmuls +0)
            wd = cst.tile([P, P], F16, tag="wd")
            nc.gpsimd.memset(wd[:], 0.0)
            xd = cst.tile([P, MM_N], F16, tag="xd")
            nc.gpsimd.memset(xd[:], 0.0)
            xts = []
            for j in range(8):
                xt = xp.tile([P, tok], F16, tag="xt")
                nc.sync.dma_start(xt[:], xt_d[j])
                xts.append(xt)

            # --- PSUM tiles + PE pacers ---
            pss = {}
            tiles = [(fb, ch) for fb in range(nfb) for ch in range(nch)]
            for fb, ch in tiles:
                pss[(fb, ch)] = pp.tile(
                    [P, MM_N], F32, tag="ps", name=f"ps{fb}_{ch}"
                )

            def pacer(stat):
                # +0 contribution: moving operand is the zeroed xd.
                # full-width: small matmuls don't refresh the p-state
                nc.tensor.matmul(pss[tiles[0]][:], stat, xd[:],
                                 start=True, stop=False)

            pacer(wd[:])  # t~1.3us: starts the PE p-state ramp epoch

            # --- decode ---
            # Sign-bit trick: ((c << (8+j)) & M) ^ (sc ^ M), with the bit
            # inversion folded into host-side sign-flip of the scales.
            ws = [None] * 8
            sgs = {}
            # per-engine no_sync dep chains pin the scheduler to the
            # intended within-queue order (it otherwise interleaves the
            # decode chains, delaying the batch-gating w tiles)
            chains = {"dve": None, "pool": None}

            def chain(key, binst):
                if key is None:
                    return
                if chains[key] is not None:
                    add_dep_helper(binst.ins, chains[key].ins, sync=False,
                                   reason="queue order")
                chains[key] = binst

            def decode_sg(j):
                sg = qp.tile([P, 8 * nfs], I16, tag="sg", name=f"sg{j}")
                nc.vector.tensor_scalar(
                    sg[:].bitcast(I32), cd[:].bitcast(I32), 8 + j, MSK,
                    OP.logical_shift_left, OP.bitwise_and,
                )
                sgs[j] = sg

            def decode_xor(j):
                wsg = qp.tile([P, 8 * nfs], I16, tag="wsg", name=f"wsg{j}")
                chains["dve1"] = nc.vector.tensor_tensor(
                    wsg[:].bitcast(I32), sgs[j][:].bitcast(I32),
                    sc_bc[:].bitcast(I32), OP.bitwise_xor,
                )
                return wsg

            def decode_tree(j, wsg, eng, chain_key=None, after=None):
                wv = wsg[:].bitcast(F16)
                h1 = qp.tile([P, 4 * nfs], F16, tag="h1", name=f"h1_{j}")
                h1_i = eng.tensor_tensor(
                    h1[:], wv[:, : 4 * nfs], wv[:, 4 * nfs:], OP.add)
                if after is not None:
                    add_dep_helper(h1_i.ins, after.ins, sync=False,
                                   reason="tree priority")
                chain(chain_key, h1_i)
                h2 = qp.tile([P, 2 * nfs], F16, tag="h2", name=f"h2_{j}")
                chain(chain_key, eng.tensor_tensor(
                    h2[:], h1[:, : 2 * nfs], h1[:, 2 * nfs:], OP.add))
                w = wp.tile([P, nfs], F16, tag="w", name=f"w{j}")
                wi = eng.tensor_tensor(w[:], h2[:, :nfs], h2[:, nfs:], OP.add)
                chain(chain_key, wi)
                ws[j] = w
                return wi

            # cd lands ~4.3us, sc ~5.8us: three TS ops (cd-only)
            # bridge the gap.  Chunk 0's tree runs on DVE first so
            # batch 0 is ready ~8.5us; Pool handles trees 1-3.
            pacer(cd[:, :P].bitcast(F16))   # ~4.4us
            for j in (0, 1, 2):
                decode_sg(j)
            pacer(sgs[0][:, :P].bitcast(F16))  # ~5.1us
            wsgs = {}
            wsgs[0] = decode_xor(0)
            wsgs[1] = decode_xor(1)   # feeds Pool's first tree early
            xor1_i = chains["dve1"]
            w0_i = decode_tree(0, wsgs[0], nc.vector, after=xor1_i)
            pacer(wsgs[0][:, :P].bitcast(F16))  # ~7.3us
            decode_tree(1, wsgs[1], nc.gpsimd, chain_key="pool")
            wsgs[2] = decode_xor(2)
            # hold XOR2 behind tree0 so the scheduler finishes the
            # batch-0-gating w tile first
            add_dep_helper(chains["dve1"].ins, w0_i.ins, sync=False,
                           reason="tree0 first")
            decode_tree(2, wsgs[2], nc.gpsimd, chain_key="pool")
            for j in range(3, 7):
                decode_sg(j)
                wsgs[j] = decode_xor(j)
                if j in POOL_TREES:
                    decode_tree(j, wsgs[j], nc.gpsimd, chain_key="pool")
                else:
                    decode_tree(j, wsgs[j], nc.vector)
            # last chunk split by fb half: half the output tiles stop,
            # evacuate and DMA out ~1.2us earlier
            decode_sg(7)
            w7h = []
            sg7v = sgs[7][:].rearrange("p (b f) -> p b f", b=BITS)
            sc7v = sc_bc[:].rearrange("p (b f) -> p b f", b=BITS)
            for h in range(nfb):
                wsg = qp.tile([P, BITS * P], I16, tag="wsg7h",
                              name=f"wsg7_{h}")
                nc.vector.tensor_tensor(
                    wsg[:].rearrange("p (b f) -> p b f", b=BITS).bitcast(I32),
                    sg7v[:, :, h * P: (h + 1) * P].bitcast(I32),
                    sc7v[:, :, h * P: (h + 1) * P].bitcast(I32),
                    OP.bitwise_xor,
                )
                wv = wsg[:].bitcast(F16)
                h1 = qp.tile([P, 4 * P], F16, tag="h17", name=f"h17_{h}")
                nc.vector.tensor_tensor(h1[:], wv[:, : 4 * P], wv[:, 4 * P:],
                                        OP.add)
                h2 = qp.tile([P, 2 * P], F16, tag="h27", name=f"h27_{h}")
                nc.vector.tensor_tensor(h2[:], h1[:, : 2 * P], h1[:, 2 * P:],
                                        OP.add)
                w = wp.tile([P, P], F16, tag="w7h", name=f"w7_{h}")
                nc.vector.tensor_tensor(w[:], h2[:, :P], h2[:, P:], OP.add)
                w7h.append(w)

            # --- matmuls in decode-completion order; two pacers bridge
            # the B0-to-B1 PE idle window (~10.2us to ~13.6us) ---
            for bi_, j in enumerate(BATCH_ORDER[:-1]):
                for ti, (fb, ch) in enumerate(tiles):
                    nc.tensor.matmul(
                        pss[(fb, ch)][:],
                        ws[j][:, fb * P: (fb + 1) * P],
                        xts[j][:, ch * MM_N: (ch + 1) * MM_N],
                        start=(j == BATCH_ORDER[0]),
                        stop=False,
                    )
                if bi_ == 0:
                    # fill the PE idle window to batch 1 (~13.6us) with
                    # +0 matmuls: an idle gap resets the p-state ramp
                    for _ in range(10):
                        nc.tensor.matmul(pss[tiles[0]][:, :256], wd[:],
                                         xd[:, :256], start=False, stop=False)

            # --- tail: last chunk tile-by-tile + evac + out DMA ---
            jl = BATCH_ORDER[-1]
            obs = []
            for ti, (fb, ch) in enumerate(tiles):
                nc.tensor.matmul(
                    pss[(fb, ch)][:],
                    w7h[fb][:],
                    xts[jl][:, ch * MM_N: (ch + 1) * MM_N],
                    start=False,
                    stop=True,
                )
                if ti % 2 == 0:
                    obp = op_.tile([P, 2 * MM_N], F16, tag="ob",
                                   name=f"ob{ti}")
                ob = obp[:, (ti % 2) * MM_N: (ti % 2 + 1) * MM_N]
                bcol = bi2[:, fb: fb + 1]
                if EVAC[ti] == "act":
                    nc.scalar.activation(ob, pss[(fb, ch)][:],
                                         ACT_F.Identity, bias=bcol, scale=1.0)
                else:
                    nc.vector.tensor_scalar(ob, pss[(fb, ch)][:],
                                            bcol, None, OP.add)
                if ti % 2 == 1:
                    obs.append((fb, ch, obp))
            # two tiles per DMA: halves the issue chains in the tail
            for pi, (fb, ch, obp) in enumerate(obs):
                deng = {"sp": nc.sync, "act": nc.scalar,
                        "pool": nc.gpsimd}[DMA_Q[pi]]
                deng.dma_start(
                    out_d[fb, ch - 1: ch + 1].rearrange("c p n -> p c n"),
                    obp[:].rearrange("p (c n) -> p c n", c=2),
                )
    nc.compile()
    return nc


_I_PERM = 8 * (np.arange(NX) % G) + np.arange(NX) // G  # i' -> i


def host_prep(x, binary, scale, bias):
    """Layout-only sharding (plus x's fp16 compute-precision cast)."""
    x2 = np.ascontiguousarray(x.reshape(N_TOK, NX).T)[_I_PERM]  # (NX, N)
    x2 = x2.astype(F16NP)  # compute dtype
    binary16 = binary.astype(np.int16)  # lossless: codes are 0..255
    in_maps = []
    for c in range(8):
        pt, pf = c // PF, c % PF
        f0 = pf * NFS
        xs = np.ascontiguousarray(x2[:, pt * TOK: (pt + 1) * TOK]).reshape(
            8, P, TOK
        )
        cs = np.ascontiguousarray(
            binary16[:, :, f0: f0 + NFS].transpose(1, 0, 2)
        ).reshape(P, 8 * NFS)
        ss = np.ascontiguousarray(
            np.broadcast_to(
                (-scale[:, f0: f0 + NFS].astype(F16NP)).reshape(1, 8 * NFS),
                (P, 8 * NFS),
            )
        )
        bs = np.ascontiguousarray(bias[f0: f0 + NFS])
        b2 = np.ascontiguousarray(bs.reshape(NFS // P, P).T)  # (128, nfb)
        in_maps.append({"xt": xs, "codes": cs, "scales": ss, "biasc": b2})
    return in_maps


def host_assemble(results):
    """results[c]["out"]: (NFB, NCH, 128, MM_N) -> full (B, T, NF)."""
    outT = np.empty((NF, N_TOK), dtype=np.float32)
    for c in range(8):
        pt, pf = c // PF, c % PF
        o = np.asarray(results[c]["out"], dtype=np.float32)
        o = o.transpose(0, 2, 1, 3).reshape(NFS, TOK)
        outT[pf * NFS: (pf + 1) * NFS, pt * TOK: (pt + 1) * TOK] = o
    return np.ascontiguousarray(outT.T).reshape(B, T, NF)


_NC_CACHE = {}


def _get_nc():
    if "nc" not in _NC_CACHE:
        nc = bacc.Bacc(None, target_bir_lowering=False)
        build_graph(nc)
        _NC_CACHE["nc"] = nc
    return _NC_CACHE["nc"]


def kernel(**inputs):
    from concourse.bass_utils import run_bass_kernel_spmd

    inputs = {k: np.asarray(v) for k, v in inputs.items()}
    in_maps = host_prep(
        inputs["x"], inputs["binary"], inputs["scale"], inputs["bias"]
    )
    res = run_bass_kernel_spmd(_get_nc(), in_maps, core_ids=list(range(8)))
    return host_assemble(res.results)


# revision 43
# speedup vs baseline: 1.0812x; 1.0405x over previous
"""BQuantConv1d Trainium2 kernel.

Math: the reference's per-token LUT + gather is algebraically a matmul:
  out[n, f] = sum_i x[n, i] * W[i, f] + bias[f]
  W[8g+j, f] = sum_b scale[b, f] * (2*bit_{7-j}(binary[b, g, f]) - 1)

Sharding: 2 token-groups x 4 f-groups over 8 cores, no collectives
(host slices inputs / concatenates outputs; layout-only host work).
Contraction order is permuted to i' = j*128 + g (host permutes xT rows
to match) so each decoded weight chunk j lands on contiguous partitions.

Per core:
  - decode W'(1024, 256) from int16 codes with a sign-bit trick:
    W element = +-scale[b, f] exactly, built by XORing the fp16 scale's
    sign bit (scales arrive sign-pre-flipped) with the masked quant bit
    (c << (8+j)) & 0x8000, as int32 SWAR (bitvec ops are DVE-only and
    32-bit-only on walrus); the fp16 b-reduction add-trees of chunks
    1-3 run on the otherwise-idle GPSIMD (Pool) engine.  no_sync dep
    edges pin the decode order so chunk completions (DVE every ~2.8us,
    Pool filling the middle ranks) match the PE's ~1.7us/batch
    consumption; the final chunk is decoded per-fb-half so half the
    output tiles finish early.
  - outT[f_shard, n_shard] = W'.T @ xT on the PE in fp16 across 8
    concurrent PSUM banks (f32), batches emitted in decode-completion
    order.  Zero-contribution pacer matmuls (junk stationary x zeroed
    moving) keep the PE p-state ramp warm through the decode phase —
    idle-gap-dependent throttling otherwise runs matmuls 2-4x slower.
  - bias is folded into the PSUM evacuation (ACT activation-bias /
    DVE tensor_scalar-bias, [P,1] per-partition operands); evacuated
    pairs share one SBUF tile so each out DMA covers two tiles (fewer
    serialized issue chains in the tail).
"""

import numpy as np

try:
    import concourse.bass as bass  # noqa: F401
except ImportError:
    import sys

    sys.path.insert(0, "/opt/trn_rl_repo")
    import concourse.bass as bass  # noqa: F401

import concourse.bacc as bacc
import concourse.mybir as mybir
import concourse.tile as tile
from concourse.tile_rust import add_dep_helper

B, T, NX, NF = 2, 2048, 1024, 1024
N_TOK = B * T
BITS = 8
G = NX // 8  # 128 code groups
PT, PF = 2, 4  # token-parallel x feature-parallel
TOK = N_TOK // PT  # tokens per core
NFS = NF // PF  # output features per core
P = 128
MM_N = 512  # moving free dim per matmul

OP = mybir.AluOpType
F32 = mybir.dt.float32
F16 = mybir.dt.float16  # compute dtype (fp16: sign-bit SWAR + more mantissa)
I16 = mybir.dt.int16
I32 = mybir.dt.int32
ACT_F = mybir.ActivationFunctionType
F16NP = np.float16

MSK = -2147450880  # 0x80008000 as int32

# chunks 1,2,3's b-reduction trees run on Pool (see decode section)
BATCH_ORDER = (0, 1, 4, 2, 5, 6, 3, 7)  # ~ decode completion order
EVAC = ("dve", "act", "dve", "act")  # per tile-pair
DMA_Q = ("sp", "sp", "act", "act")  # one queue per tile-pair DMA


def build_graph(nc, tok=TOK, nfs=NFS):
    nfb = nfs // P  # f blocks of 128
    nch = tok // MM_N  # moving chunks
    xt_d = nc.dram_tensor("xt", (8, P, tok), F16, kind="ExternalInput")
    cd_d = nc.dram_tensor("codes", (P, 8 * nfs), I16, kind="ExternalInput")
    sc_d = nc.dram_tensor("scales", (P, 8 * nfs), F16, kind="ExternalInput")
    b2_d = nc.dram_tensor("biasc", (P, nfs // P), F32, kind="ExternalInput")
    out_d = nc.dram_tensor("out", (nfb, nch, P, MM_N), F16, kind="ExternalOutput")

    with tile.TileContext(nc) as tc:
        with (
            tc.tile_pool(name="xp", bufs=8) as xp,
            tc.tile_pool(name="cp", bufs=1) as cp,
            tc.tile_pool(name="wp", bufs=8) as wp,
            tc.tile_pool(name="qp", bufs=8) as qp,
            tc.tile_pool(name="cst", bufs=1) as cst,
            tc.tile_pool(name="op", bufs=8) as op_,
            tc.tile_pool(name="pp", bufs=8, space="PSUM") as pp,
        ):
            # --- loads; codes first: decode is the critical path ---
            cd = cp.tile([P, 8 * nfs], I16, tag="cd")
            nc.sync.dma_start(cd[:], cd_d[:])
            sc_bc = cst.tile([P, 8 * nfs], F16, tag="sc_bc")
            nc.scalar.dma_start(sc_bc[:], sc_d[:])
            bi2 = cst.tile([P, nfb], F32, tag="bi2")
            nc.scalar.dma_start(bi2[:], b2_d[:])
            # PE pacer scratch (zeroed moving makes pacer matmuls +0)
            wd = cst.tile([P, P], F16, tag="wd")
            nc.gpsimd.memset(wd[:], 0.0)
            xd = cst.tile([P, MM_N], F16, tag="xd")
            nc.gpsimd.memset(xd[:], 0.0)
            xts = []
            for j in range(8):
                xt = xp.tile([P, tok], F16, tag="xt")
                nc.sync.dma_start(xt[:], xt_d[j])
                xts.append(xt)

            # --- PSUM tiles + PE pacers ---
            pss = {}
            tiles = [(fb, ch) for fb in range(nfb) for ch in range(nch)]
            for fb, ch in tiles:
                pss[(fb, ch)] = pp.tile(
                    [P, MM_N], F32, tag="ps", name=f"ps{fb}_{ch}"
                )[:]

            def pacer(stat):
                # +0 contribution: moving operand is the zeroed xd.
                # full-width: small matmuls don't refresh the p-state
                nc.tensor.matmul(pss[tiles[0]], stat, xd[:],
                                 start=True, stop=False)

            pacer(wd[:])  # t~1.3us: starts the PE p-state ramp epoch

            # --- decode ---
            # Sign-bit trick: ((c << (8+j)) & M) ^ (sc ^ M), with the bit
            # inversion folded into host-side sign-flip of the scales.
            ws = [None] * 8
            sgs = {}
            # per-engine no_sync dep chains pin the scheduler to the
            # intended within-queue order (it otherwise interleaves the
            # decode chains, delaying the batch-gating w tiles)
            chains = {"dve": None, "pool": None}

            def chain(key, binst):
                if key is None:
                    return
                if chains[key] is not None:
                    add_dep_helper(binst.ins, chains[key].ins, sync=False,
                                   reason="queue order")
                chains[key] = binst

            def decode_sg(j):
                sg = qp.tile([P, 8 * nfs], I16, tag="sg", name=f"sg{j}")
                nc.vector.tensor_scalar(
                    sg[:].bitcast(I32), cd[:].bitcast(I32), 8 + j, MSK,
                    OP.logical_shift_left, OP.bitwise_and,
                )
                sgs[j] = sg

            def decode_xor(j, after=None):
                wsg = qp.tile([P, 8 * nfs], I16, tag="wsg", name=f"wsg{j}")
                xi = nc.vector.tensor_tensor(
                    wsg[:].bitcast(I32), sgs[j][:].bitcast(I32),
                    sc_bc[:].bitcast(I32), OP.bitwise_xor,
                )
                if after is not None:
                    add_dep_helper(xi.ins, after.ins, sync=False,
                                   reason="dve order")
                return wsg, xi

            def decode_tree(j, wsg, eng, chain_key=None, after=None):
                wv = wsg[:].bitcast(F16)
                h1 = qp.tile([P, 4 * nfs], F16, tag="h1", name=f"h1_{j}")
                h1_i = eng.tensor_tensor(
                    h1[:], wv[:, : 4 * nfs], wv[:, 4 * nfs:], OP.add)
                if after is not None:
                    add_dep_helper(h1_i.ins, after.ins, sync=False,
                                   reason="tree priority")
                chain(chain_key, h1_i)
                h2 = qp.tile([P, 2 * nfs], F16, tag="h2", name=f"h2_{j}")
                chain(chain_key, eng.tensor_tensor(
                    h2[:], h1[:, : 2 * nfs], h1[:, 2 * nfs:], OP.add))
                w = wp.tile([P, nfs], F16, tag="w", name=f"w{j}")
                wi = eng.tensor_tensor(w[:], h2[:, :nfs], h2[:, nfs:], OP.add)
                chain(chain_key, wi)
                ws[j] = w
                return wi

            # cd lands ~4.3us, sc ~5.8us: three TS ops (cd-only)
            # bridge the gap.  Chunk 0's tree runs on DVE first so
            # batch 0 is ready ~8.5us; Pool handles trees 1-3.
            pacer(cd[:, :P].bitcast(F16))   # ~4.4us
            for j in (0, 1, 2):
                decode_sg(j)
            pacer(sgs[0][:, :P].bitcast(F16))  # ~5.1us
            # DVE order (dep-pinned): tree0 -> chunk4 -> XOR2(Pool T2)
            # -> chunk5 -> XOR3(Pool T3) -> chunk6 -> chunk7(split):
            # DVE completions land ~every 2.8us, Pool fills the middle
            # ranks, matching the PE's 1.7us/batch consumption.
            wsgs = {}
            wsgs[1], xor1_i = decode_xor(1)  # feeds Pool's first tree
            wsgs[0], xor0_i = decode_xor(0)
            w0_i = decode_tree(0, wsgs[0], nc.vector, after=xor1_i)
            pacer(wsgs[0][:, :P].bitcast(F16))  # ~7.3us
            decode_tree(1, wsgs[1], nc.gpsimd, chain_key="pool")
            wsgs[2], xor2_i = decode_xor(2, after=w0_i)
            decode_tree(2, wsgs[2], nc.gpsimd, chain_key="pool")
            decode_sg(4)
            wsgs[4], xor4_i = decode_xor(4)
            w4_i = decode_tree(4, wsgs[4], nc.vector)
            decode_sg(6)
            wsgs[6], xor6_i = decode_xor(6, after=w4_i)
            decode_tree(6, wsgs[6], nc.gpsimd, chain_key="pool")
            decode_sg(5)
            wsgs[5], xor5_i = decode_xor(5)
            w5_i = decode_tree(5, wsgs[5], nc.vector)
            decode_sg(3)
            wsgs[3], xor3_i = decode_xor(3, after=w5_i)
            w6_i = decode_tree(3, wsgs[3], nc.vector)
            # last chunk split by fb half: half the output tiles stop,
            # evacuate and DMA out ~1.2us earlier
            decode_sg(7)
            w7h = []
            hold = w6_i
            sg7v = sgs[7][:].rearrange("p (b f) -> p b f", b=BITS)
            sc7v = sc_bc[:].rearrange("p (b f) -> p b f", b=BITS)
            for h in range(nfb):
                wsg = qp.tile([P, BITS * P], I16, tag="wsg7h",
                              name=f"wsg7_{h}")
                x7i = nc.vector.tensor_tensor(
                    wsg[:].rearrange("p (b f) -> p b f", b=BITS).bitcast(I32),
                    sg7v[:, :, h * P: (h + 1) * P].bitcast(I32),
                    sc7v[:, :, h * P: (h + 1) * P].bitcast(I32),
                    OP.bitwise_xor,
                )
                add_dep_helper(x7i.ins, hold.ins, sync=False,
                               reason="dve order")
                hold = x7i
                wv = wsg[:].bitcast(F16)
                h1 = qp.tile([P, 4 * P], F16, tag="h17", name=f"h17_{h}")
                nc.vector.tensor_tensor(h1[:], wv[:, : 4 * P], wv[:, 4 * P:],
                                        OP.add)
                h2 = qp.tile([P, 2 * P], F16, tag="h27", name=f"h27_{h}")
                nc.vector.tensor_tensor(h2[:], h1[:, : 2 * P], h1[:, 2 * P:],
                                        OP.add)
                w = wp.tile([P, P], F16, tag="w7h", name=f"w7_{h}")
                nc.vector.tensor_tensor(w[:], h2[:, :P], h2[:, P:], OP.add)
                w7h.append(w)

            # --- matmuls in decode-completion order; two pacers bridge
            # the B0-to-B1 PE idle window (~10.2us to ~13.6us) ---
            for bi_, j in enumerate(BATCH_ORDER[:-1]):
                for ti, (fb, ch) in enumerate(tiles):
                    nc.tensor.matmul(
                        pss[(fb, ch)],
                        ws[j][:, fb * P: (fb + 1) * P],
                        xts[j][:, ch * MM_N: (ch + 1) * MM_N],
                        start=(j == BATCH_ORDER[0]),
                        stop=False,
                    )
                if bi_ == 0:
                    # fill the PE idle window to batch 1 (~13.6us) with
                    # +0 matmuls: an idle gap resets the p-state ramp
                    for _ in range(6):
                        nc.tensor.matmul(pss[tiles[0]][:, :256], wd[:],
                                         xd[:, :256], start=False, stop=False)

            # --- tail: last chunk tile-by-tile + evac + out DMA ---
            jl = BATCH_ORDER[-1]
            obs = []
            for ti, (fb, ch) in enumerate(tiles):
                nc.tensor.matmul(
                    pss[(fb, ch)][:],
                    w7h[fb][:],
                    xts[jl][:, ch * MM_N: (ch + 1) * MM_N],
                    start=False,
                    stop=True,
                )
                if ti % 2 == 1:
                    # one 1024-wide evac per adjacent-tile pair; both
                    # tiles share fb, so one [P,1] bias operand serves
                    obp = op_.tile([P, 2 * MM_N], F16, tag="ob",
                                   name=f"ob{ti}")
                    psv = pss[(fb, ch - 1)], pss[(fb, ch)]
                    bcol = bi2[:, fb: fb + 1]
                    pi = ti // 2
                    for half, ps in enumerate(psv):
                        ob = obp[:, half * MM_N: (half + 1) * MM_N]
                        if EVAC[pi] == "act" and half == 0:
                            pass
                    if EVAC[pi] == "act":
                        nc.scalar.activation(
                            obp[:, :MM_N], psv[0][:], ACT_F.Identity,
                            bias=bcol, scale=1.0)
                        nc.scalar.activation(
                            obp[:, MM_N:], psv[1][:], ACT_F.Identity,
                            bias=bcol, scale=1.0)
                    else:
                        nc.vector.tensor_scalar(
                            obp[:, :MM_N], psv[0][:], bcol, None, OP.add)
                        nc.vector.tensor_scalar(
                            obp[:, MM_N:], psv[1][:], bcol, None, OP.add)
                    obs.append((fb, ch, obp))
            # two tiles per DMA: halves the issue chains in the tail
            for pi, (fb, ch, obp) in enumerate(obs):
                deng = {"sp": nc.sync, "act": nc.scalar,
                        "pool": nc.gpsimd}[DMA_Q[pi]]
                deng.dma_start(
                    out_d[fb, ch - 1: ch + 1].rearrange("c p n -> p c n"),
                    obp[:].rearrange("p (c n) -> p c n", c=2),
                )
    nc.compile()
    return nc


_I_PERM = 8 * (np.arange(NX) % G) + np.arange(NX) // G  # i' -> i


def host_prep(x, binary, scale, bias):
    """Layout-only sharding (plus x's fp16 compute-precision cast)."""
    x2 = np.ascontiguousarray(x.reshape(N_TOK, NX).T)[_I_PERM]  # (NX, N)
    x2 = x2.astype(F16NP)  # compute dtype
    binary16 = binary.astype(np.int16)  # lossless: codes are 0..255
    in_maps = []
    for c in range(8):
        pt, pf = c // PF, c % PF
        f0 = pf * NFS
        xs = np.ascontiguousarray(x2[:, pt * TOK: (pt + 1) * TOK]).reshape(
            8, P, TOK
        )
        cs = np.ascontiguousarray(
            binary16[:, :, f0: f0 + NFS].transpose(1, 0, 2)
        ).reshape(P, 8 * NFS)
        ss = np.ascontiguousarray(
            np.broadcast_to(
                (-scale[:, f0: f0 + NFS].astype(F16NP)).reshape(1, 8 * NFS),
                (P, 8 * NFS),
            )
        )
        bs = np.ascontiguousarray(bias[f0: f0 + NFS])
        b2 = np.ascontiguousarray(bs.reshape(NFS // P, P).T)  # (128, nfb)
        in_maps.append({"xt": xs, "codes": cs, "scales": ss, "biasc": b2})
    return in_maps


def host_assemble(results):
    """results[c]["out"]: (NFB, NCH, 128, MM_N) -> full (B, T, NF)."""
    outT = np.empty((NF, N_TOK), dtype=np.float32)
    for c in range(8):
        pt, pf = c // PF, c % PF
        o = np.asarray(results[c]["out"], dtype=np.float32)
        o = o.transpose(0, 2, 1, 3).reshape(NFS, TOK)
        outT[pf * NFS: (pf + 1) * NFS, pt * TOK: (pt + 1) * TOK] = o
    return np.ascontiguousarray(outT.T).reshape(B, T, NF)


_NC_CACHE = {}


def _get_nc():
    if "nc" not in _NC_CACHE:
        nc = bacc.Bacc(None, target_bir_lowering=False)
        build_graph(nc)
        _NC_CACHE["nc"] = nc
    return _NC_CACHE["nc"]


def kernel(**inputs):
    from concourse.bass_utils import run_bass_kernel_spmd

    inputs = {k: np.asarray(v) for k, v in inputs.items()}
    in_maps = host_prep(
        inputs["x"], inputs["binary"], inputs["scale"], inputs["bias"]
    )
    res = run_bass_kernel_spmd(_get_nc(), in_maps, core_ids=list(range(8)))
    return host_assemble(res.results)
